# revision 18
# baseline (speedup 1.0000x reference)
"""Trainium2 Bass kernel for nn_AttentionBlock (B=2,S=2048,D=1024,H=16,MLP=4096).

Structure: the reference does q.reshape(B,H,S,HD) on a row-major [B,S,D]
tensor, so head hh consumes ROWS 128*hh:128*(hh+1) of the projected
[2048,1024] matrix reinterpreted as [2048,64]. The block decomposes into 32
independent 128-row blocks (B*H) aligned across all ops. Sharding: 8 cores x
4 head-blocks (512 rows) each, zero collectives.

v3 design (from the v2 HW trace):
  - Startup: scalar queue carries NO DMAs (v2 jammed it with 47us of
    descriptor-gen, delaying the ACT table load + LN1 to 52us). All loads
    go on gpsimd (weights/masks/consts) and sync (x, staging); first QKV
    matmul ~13us, phase B ~30us.
  - Masks host-packed to [128,16,2048] so each e-group is one flat
    contiguous-per-partition DMA (4 total).
  - Scores: K=64 matmuls row-paired via PE 32-row tiling. kblk2 keeps
    even-e K vectors on partitions 0:64 and odd-e on 64:128 (written
    directly from the projection PSUM halves - no staging DMA); qblk2 is
    duplicated into both partition halves (2 bulk SBUF copies). Two
    concurrent K=64 tiles -> ~2x score throughput.
  - AV augmented with a ones-column (col 64) and zero-pad to M=80 so the
    per-unit [80,1024] outp transposes via the DMA xbar (p%16==0) on the
    otherwise-idle sync queue. PE transposes gone.
  - All mask multiplies on DVE (v2 split them DVE/GpSimd, but the engines
    share one SBUF port: measured DVE TT 1846ns vs 594 solo, GpSimd 2120).
  - Phase B paces on the scalar exp stream (~145us): PE ~83us busy, DVE
    ~110us, sync ~18us, gpsimd only w1/w2 prefetch descriptors.
  - rsqrt for both layernorms via exp(-0.5*ln(var+eps)): Ln and Exp share
    one ACT table set -> zero table reloads inside the exp stream.
"""
import sys

sys.path.insert(0, "/opt/trn_rl_repo")

import numpy as np
import ml_dtypes

import concourse.bass as bass
import concourse.mybir as mybir
import concourse.tile as tile
from concourse.bass_utils import run_bass_kernel_spmd

B, S, D = 2, 2048, 1024
H, HD, MLP = 16, 64, 4096
EPS = 1e-5
R = 512          # rows per core
NCORES = 8
F32 = mybir.dt.float32
BF16 = mybir.dt.bfloat16
FP8 = mybir.dt.float8e4
BF = ml_dtypes.bfloat16
F8NP = ml_dtypes.float8_e4m3fn
AF = mybir.ActivationFunctionType
AOP = mybir.AluOpType
DR = mybir.MatmulPerfMode.DoubleRow

WQ_SCALE = 256.0   # host multiplies wq/8 by this before fp8 quantization
WK_SCALE = 32.0
WV_SCALE = 32.0

# cpack column offsets (all [128, n] with vec[128*t+p] at [p, t])
C_BQ, C_BK, C_G1, C_B1, C_G2, C_B2L, C_B1M = 0, 8, 16, 24, 32, 40, 48


# walrus in this container rejects >1 sync-wait on TPB_CTRL (Drain): split the
# TileContext final-drain waits across sequential drains (same AND semantics).
def _patch_drain():
    if getattr(tile.TileContext, "_dab_patched", False):
        return

    def _patched_dab(self, tick_clock, wait_clock):
        from concourse.vector_clock import ScopedClock
        drain_inst = self.nc.sync.drain()
        wait_clock.add_sem_waits(drain_inst.ins,
                                 ScopedClock({None: tick_clock.global_clock}))
        si = drain_inst.ins.sync_info
        if si is not None and len(si.on_wait) > 1:
            waits = list(si.on_wait)
            drain_inst.ins.sync_info = mybir.SyncInfo(on_wait=waits[:1],
                                                      on_update=list(si.on_update))
            for w in waits[1:]:
                extra = self.nc.sync.drain()
                extra.ins.sync_info = mybir.SyncInfo(on_wait=[w], on_update=[])
        self.nc.all_engine_barrier()
        assert self.sems is not None
        popped = self.nc._tile_sem_poison_stack.pop()
        assert popped is self._sem_poison
        self.nc.clear_and_free_semaphores(list(self.sems.allocated().values()))
        self.nc.all_engine_barrier()

    tile.TileContext._drain_and_barrier = _patched_dab
    tile.TileContext._dab_patched = True


# This walrus build accepts at most ONE sync-wait per instruction: hoist
# excess waits onto same-engine NoOp carriers placed immediately before the
# instruction (engine streams execute in order, so semantics are preserved).
_WAIT_LIMIT = 1


def _split_waits(nc):
    n_carriers = 0
    for bbname, bbw in nc.bb_map.items():
        il = bbw.bb.instructions
        out = []
        for inst in il:
            si = inst.sync_info
            if si is not None and len(si.on_wait) > _WAIT_LIMIT:
                waits = list(si.on_wait)
                extra, keep = waits[:-_WAIT_LIMIT], waits[-_WAIT_LIMIT:]
                for w in extra:
                    nop = mybir.InstNoOp(name=f"wsplit_{n_carriers}", ins=[], outs=[])
                    nop.engine = inst.engine
                    nop.sync_info = mybir.SyncInfo(on_wait=[w], on_update=[])
                    nc.register_instruction(nop, overwrite=True)
                    out.append(nop)
                    n_carriers += 1
                inst.sync_info = mybir.SyncInfo(on_wait=keep,
                                                on_update=list(si.on_update))
            out.append(inst)
        bbw.bb.instructions = out
    return n_carriers


def _rsqrt(nc, pool, mv, eps_t, name):
    """1/sqrt(var+eps) via exp(-0.5*ln(var+eps)) — stays on the ln/exp ACT
    table set, so it never forces a table reload inside the exp stream."""
    lg = pool.tile([128, 1], F32, tag="lg", name=f"lg_{name}")
    nc.scalar.activation(lg, mv[:, 1:2], AF.Ln, bias=eps_t, scale=1.0)
    rs = pool.tile([128, 1], F32, tag="rs", name=f"rs_{name}")
    nc.scalar.activation(rs, lg, AF.Exp, scale=-0.5)
    return rs


def _phase_a(nc, tc, consts, dram):
    """LN1 -> hT8 (fp8), QKV DoubleRow projections -> qblk2/kblk2/vaug."""
    eps_t = consts["eps"]
    x_sb = consts["x_sb"]
    cpack = consts["cpack"]
    qblk2, kblk2, vaug = consts["qblk2"], consts["kblk2"], consts["vaug"]
    with tc.tile_pool(name="phA", bufs=1) as pA, \
         tc.tile_pool(name="stat", bufs=8) as stat, \
         tc.tile_pool(name="psP", bufs=4, space="PSUM") as psP, \
         tc.tile_pool(name="stg", bufs=2) as stg:
        xc = pA.tile([128, 4, D], BF16, tag="xc")
        hT = pA.tile([128, 8, R], BF16, tag="hT")
        hT8 = pA.tile([128, 8, R], FP8, tag="hT8")
        wq8f = pA.tile([128, 8192], FP8, tag="wq8")
        wk8f = pA.tile([128, 8192], FP8, tag="wk8")
        wv8f = pA.tile([128, 8192], FP8, tag="wv8")
        wq8 = wq8f.rearrange("p (t8 c t m) -> p t8 c t m", t8=8, c=4, t=2)
        wk8 = wk8f.rearrange("p (t8 c t m) -> p t8 c t m", t8=8, c=4, t=2)
        wv8 = wv8f.rearrange("p (c t d) -> p c t d", c=4, t=2)
        x_r = dram["x"].rearrange("(n p) d -> n p d", p=128)
        # sync queue: x chunks first
        for i in range(4):
            nc.sync.dma_start(x_sb[:, i, :], x_r[i])
        # HAM warm-up: the PE clock-gate defaults to 4/8 (1.2 GHz) and only
        # releases after ~3.4us of sustained matmul activity; without this,
        # QKV and early phase B run at half clock (~40us penalty). Dummy
        # matmuls on already-loaded data bridge the DMA-bound startup.
        warm = psP.tile([128, 512], F32, tag="pp", name="warm_ps")
        for i in range(60):
            nc.tensor.matmul(warm, x_sb[:, 0, 0:128], x_sb[:, 0, 0:512],
                             start=True, stop=True)
        # gpsimd queue: weights -> small consts -> masks (order = need order)
        nc.gpsimd.dma_start(wk8f, dram["wk"][:, :])
        nc.gpsimd.dma_start(wq8f, dram["wq"][:, :])
        nc.gpsimd.dma_start(consts["cpack"], dram["cpack"][:, :])
        nc.gpsimd.dma_start(consts["bvg"], dram["bvg"][:, :])
        nc.gpsimd.dma_start(wv8f, dram["wv"][:, :])
        # LN1 (scalar queue has only the ACT table load ahead of these)
        for i in range(4):
            st = stat.tile([128, 2, 6], F32, tag="st")
            nc.vector.bn_stats(st[:, 0, :], x_sb[:, i, 0:512])
            nc.vector.bn_stats(st[:, 1, :], x_sb[:, i, 512:D])
            mv = stat.tile([128, 2], F32, tag="mv")
            nc.vector.bn_aggr(mv, st)
            rs = _rsqrt(nc, stat, mv, eps_t, f"a{i}")
            nc.vector.tensor_scalar(out=xc[:, i, :], in0=x_sb[:, i, :],
                                    scalar1=mv[:, 0:1], scalar2=rs,
                                    op0=AOP.subtract, op1=AOP.mult)
            nc.sync.dma_start(hT[:, :, 128 * i:128 * i + 128],
                              xc[:, i, :], transpose=True)
        warm2 = psP.tile([128, 512], F32, tag="pp", name="warm_ps2")
        for i in range(12):
            nc.tensor.matmul(warm2, hT[:, 0, 0:128], hT[:, 0, :],
                             start=True, stop=True)
        for dt8 in range(8):
            nc.vector.tensor_scalar(out=hT8[:, dt8, :], in0=hT[:, dt8, :],
                                    scalar1=cpack[:, C_G1 + dt8:C_G1 + dt8 + 1],
                                    scalar2=cpack[:, C_B1 + dt8:C_B1 + dt8 + 1],
                                    op0=AOP.mult, op1=AOP.add)
        # Mask loads: 8MB of HBM traffic that would starve the startup-
        # critical hT transposes off the DMA engines. The marker DMA below
        # reads hT8 (which depends on all four transposes) and sits on the
        # same gpsimd queue as the mask DMAs, so queue order holds the mask
        # traffic back until the leg-1 transpose chain is done (~26us).
        # (The real g=0 mask DMA overwrites the marker's 4 bytes after.)
        nc.gpsimd.dma_start(consts["mask_sb"][0:1, 0:1, 0:4],
                            hT[0:1, 7:8, 508:512])
        for g in range(4):
            nc.gpsimd.dma_start(consts["mask_sb"][:, 4 * g:4 * g + 4, :],
                                dram["maskp"][:, 4 * g:4 * g + 4, :])
        # K projection: dequant writes kblk2 halves directly (even e on
        # partitions 0:64, odd e on 64:128) - no staging DMA.
        for t8 in range(8):
            ps = psP.tile([128, R], F32, tag="pp", name=f"ps_k_{t8}")
            for ci in range(4):
                nc.tensor.matmul(ps, wk8[:, t8, ci, :, :],
                                 hT8[:, 2 * ci:2 * ci + 2, :],
                                 start=(ci == 0), stop=(ci == 3),
                                 perf_mode=DR)
            for half in range(2):
                sl = slice(64 * half, 64 * half + 64)
                nc.vector.tensor_scalar(
                    out=kblk2[sl, :, t8, :],
                    in0=ps[sl, :].rearrange("p (j r) -> p j r", j=4),
                    scalar1=1.0 / WK_SCALE,
                    scalar2=cpack[sl, C_BK + t8:C_BK + t8 + 1],
                    op0=AOP.mult, op1=AOP.add)
        # Q projection -> sg staging tile -> qblk2 lower half via sync DMAs
        for t8 in range(8):
            ps = psP.tile([128, R], F32, tag="pp", name=f"ps_q_{t8}")
            for ci in range(4):
                nc.tensor.matmul(ps, wq8[:, t8, ci, :, :],
                                 hT8[:, 2 * ci:2 * ci + 2, :],
                                 start=(ci == 0), stop=(ci == 3),
                                 perf_mode=DR)
            sg = stg.tile([128, R], BF16, tag="sg", name=f"sg_q_{t8}")
            nc.vector.tensor_scalar(out=sg, in0=ps, scalar1=1.0 / WQ_SCALE,
                                    scalar2=cpack[:, C_BQ + t8:C_BQ + t8 + 1],
                                    op0=AOP.mult, op1=AOP.add)
            nc.sync.dma_start(qblk2[0:64, :, 2 * t8, :],
                              sg[0:64, :].rearrange("p (j r) -> p j r", j=4))
            nc.sync.dma_start(qblk2[0:64, :, 2 * t8 + 1, :],
                              sg[64:128, :].rearrange("p (j r) -> p j r", j=4))
            if t8 == 3:
                nc.sync.dma_start(qblk2[64:128, :, 0:8, :],
                                  qblk2[0:64, :, 0:8, :])
        nc.sync.dma_start(qblk2[64:128, :, 8:16, :], qblk2[0:64, :, 8:16, :])
        # V projection (natural layout) -> vaug cols 0:64
        for rt in range(4):
            for hf in range(2):
                ps = psP.tile([128, R], F32, tag="pp", name=f"ps_v_{rt}_{hf}")
                for ci in range(4):
                    nc.tensor.matmul(ps, hT8[:, 2 * ci:2 * ci + 2,
                                             128 * rt:128 * rt + 128],
                                     wv8[:, ci, :, 512 * hf:512 * hf + 512],
                                     start=(ci == 0), stop=(ci == 3),
                                     perf_mode=DR)
                nc.vector.tensor_scalar(
                    out=vaug[:, rt, 8 * hf:8 * hf + 8, 0:64],
                    in0=ps.rearrange("p (e dd) -> p e dd", dd=64),
                    scalar1=1.0 / WV_SCALE, scalar2=None, op0=AOP.mult)


def _phase_b(nc, tc, consts, dram, hattn, xc2, h2T):
    """Attention paced by the exp-ACT stream + per-j LN2.

    Units u = 2*j + hh over (head-block j, column half hh). Per unit, 8
    slots each compute scores for (e=2s, e=2s+1) as two concurrent PE
    row-tiles, exp+mask them, and interleave the PREVIOUS unit's AV
    accumulation; then the previous unit's normalize (DVE copy, sync-queue
    DMA transpose, reciprocal, scale) and, per odd unit, LN2."""
    qblk2, kblk2, vaug, mask_sb = (consts["qblk2"], consts["kblk2"],
                                   consts["vaug"], consts["mask_sb"])
    eps_t = consts["eps"]
    x_sb = consts["x_sb"]
    with tc.tile_pool(name="exB", bufs=18) as exp_pool, \
         tc.tile_pool(name="nmB", bufs=2) as nms, \
         tc.tile_pool(name="tpB", bufs=3) as tps, \
         tc.tile_pool(name="attst", bufs=8) as attst, \
         tc.tile_pool(name="stat2", bufs=8) as stat2, \
         tc.tile_pool(name="tmp2", bufs=2) as tmp2, \
         tc.tile_pool(name="opB", bufs=1, space="PSUM") as outps, \
         tc.tile_pool(name="scB", bufs=3, space="PSUM") as scps:
        exs = {}     # (u, e) -> ex tile
        outp = {}    # u -> outp psum tile

        def emit_pass2_slot(u, e):
            j = u // 2
            for c2 in range(2):
                nc.tensor.matmul(outp[u][:, 512 * c2:512 * c2 + 512],
                                 vaug[:, j, e, :],
                                 exs[(u, e)][:, 512 * c2:512 * c2 + 512],
                                 start=(e == 0), stop=(e == 15))

        def emit_norm(u):
            j, hh = divmod(u, 2)
            nm = nms.tile([80, 1024], BF16, tag="nm", name=f"nm_{u}")
            nc.vector.tensor_copy(nm, outp[u])
            tpb = tps.tile([128, 8, 80], BF16, tag="tpb", name=f"tpb_{u}")
            nc.sync.dma_start(tpb, nm, transpose=True)
            for lc in range(8):
                c16 = 8 * hh + lc
                rd = attst.tile([128, 1], F32, tag="rd", name=f"rd_{u}_{lc}")
                nc.vector.reciprocal(rd, tpb[:, lc, 64:65])
                nc.vector.tensor_scalar(
                    out=hattn[:, j, 64 * c16:64 * c16 + 64],
                    in0=tpb[:, lc, 0:64], scalar1=rd,
                    scalar2=None, op0=AOP.mult)

        def emit_ln2(j):
            h2p = tmp2.tile([128, D], F32, tag="h2p", name=f"h2p_{j}")
            nc.vector.tensor_tensor(out=h2p, in0=hattn[:, j, :],
                                    in1=x_sb[:, j, :], op=AOP.add)
            nc.vector.tensor_tensor(out=h2p, in0=h2p, in1=consts["bv_b"],
                                    op=AOP.add)
            st = stat2.tile([128, 2, 6], F32, tag="st", name=f"st2_{j}")
            nc.vector.bn_stats(st[:, 0, :], h2p[:, 0:512])
            nc.vector.bn_stats(st[:, 1, :], h2p[:, 512:D])
            mv = stat2.tile([128, 2], F32, tag="mv", name=f"mv2_{j}")
            nc.vector.bn_aggr(mv, st)
            rs = _rsqrt(nc, stat2, mv, eps_t, f"b{j}")
            nc.vector.tensor_scalar(out=xc2[:, j, :], in0=h2p,
                                    scalar1=mv[:, 0:1], scalar2=rs,
                                    op0=AOP.subtract, op1=AOP.mult)
            # fold the ln2 gamma here (b2l is host-folded into b1m/bcomb):
            # xc2 becomes g2*normalized, so CD needs no h2T affine pass.
            nc.vector.tensor_tensor(out=xc2[:, j, :], in0=xc2[:, j, :],
                                    in1=consts["g2_b"], op=AOP.mult)
            nc.sync.dma_start(h2T[:, :, 128 * j:128 * j + 128],
                              xc2[:, j, :], transpose=True)

        for u in range(8):
            j, hh = divmod(u, 2)
            outp[u] = outps.tile([80, 1024], F32, tag="op", name=f"op_{u}")
            for s in range(8):
                sc = {}
                for par in range(2):
                    sc[par] = scps.tile([128, 1024], F32, tag="sc",
                                        name=f"sc_{u}_{2 * s + par}")
                # ABAB interleave: adjacent MMs target different PE row
                # groups so the 32-row tiling can overlap them.
                for cq in range(2):
                    for par in range(2):
                        sl = slice(64 * par, 64 * par + 64)
                        nc.tensor.matmul(
                            sc[par][:, 512 * cq:512 * cq + 512],
                            kblk2[sl, j, s, :],
                            qblk2[sl, j, 8 * hh + 4 * cq:8 * hh + 4 * cq + 4, :],
                            start=True, stop=True)
                for par in range(2):
                    e = 2 * s + par
                    ex_ = exp_pool.tile([128, 1024], BF16, tag="ex",
                                        name=f"ex_{u}_{e}")
                    exs[(u, e)] = ex_
                    nc.scalar.activation(ex_, sc[par], AF.Exp)
                    nc.vector.tensor_tensor(
                        out=ex_, in0=ex_,
                        in1=mask_sb[:, e, 1024 * hh:1024 * hh + 1024],
                        op=AOP.mult)
                    if u > 0:
                        emit_pass2_slot(u - 1, 2 * s + par)
            if u > 0:
                emit_norm(u - 1)
                if (u - 1) % 2 == 1:
                    emit_ln2((u - 1) // 2)
                for e in range(16):
                    exs.pop((u - 1, e), None)
        for e in range(16):
            emit_pass2_slot(7, e)
        emit_norm(7)
        emit_ln2(3)


def _phase_cd(nc, tc, consts, dram, xc2, h2T):
    """h2T affine; MLP; final residual; DMA out."""
    cpack = consts["cpack"]
    with tc.tile_pool(name="phC", bufs=1) as pC:
        ghT = pC.tile([128, 32, R], BF16, tag="ghT")
        with tc.tile_pool(name="w1l", bufs=2) as w1l, \
             tc.tile_pool(name="psD1", bufs=4, space="PSUM") as psD1:
            for mt4 in range(8):
                w1t = w1l.tile([128, 4, 8, 128], BF16, tag="w1", name=f"w1t_{mt4}")
                nc.gpsimd.dma_start(
                    w1t, dram["w1"][4 * mt4:4 * mt4 + 4].rearrange("m p c n -> p m c n"))
                for sub in range(4):
                    mt = 4 * mt4 + sub
                    ps = psD1.tile([128, R], F32, tag="m1", name=f"m1_{mt}")
                    for ci in range(8):
                        nc.tensor.matmul(ps, w1t[:, sub, ci, :], h2T[:, ci, :],
                                         start=(ci == 0), stop=(ci == 7))
                    nc.scalar.activation(ghT[:, mt, :], ps, AF.Gelu,
                                         bias=cpack[:, C_B1M + mt:C_B1M + mt + 1],
                                         scale=1.0)
        resf = pC.tile([128, 4, D], F32, tag="resf")
        for rt in range(4):
            nc.vector.tensor_tensor(out=resf[:, rt, :], in0=xc2[:, rt, :],
                                    in1=consts["bcomb_b"], op=AOP.add)
        ostg = pC.tile([128, 4, D], F32, tag="ostg")
        out_r = dram["out"].rearrange("(n p) d -> n p d", p=128)
        with tc.tile_pool(name="w2l", bufs=2) as w2l, \
             tc.tile_pool(name="psD2", bufs=1, space="PSUM") as psD2:
            m2ps = [[psD2.tile([128, 512], F32, tag=f"m2_{rt}_{dc}",
                               name=f"m2_{rt}_{dc}")
                     for dc in range(2)] for rt in range(4)]
            for mc4 in range(8):
                w2t = w2l.tile([128, 4, D], BF16, tag="w2", name=f"w2t_{mc4}")
                nc.gpsimd.dma_start(
                    w2t, dram["w2"][512 * mc4:512 * mc4 + 512, :].rearrange(
                        "(m p) d -> p m d", p=128))
                for sub in range(4):
                    mc = 4 * mc4 + sub
                    for rt in range(4):
                        for dc in range(2):
                            nc.tensor.matmul(m2ps[rt][dc],
                                             ghT[:, mc, 128 * rt:128 * rt + 128],
                                             w2t[:, sub, 512 * dc:512 * dc + 512],
                                             start=(mc == 0), stop=(mc == 31))
            for rt in range(4):
                for dc in range(2):
                    nc.vector.tensor_tensor(out=ostg[:, rt, 512 * dc:512 * dc + 512],
                                            in0=m2ps[rt][dc],
                                            in1=resf[:, rt, 512 * dc:512 * dc + 512],
                                            op=AOP.add)
                nc.sync.dma_start(out_r[rt], ostg[:, rt, :])


def _build_program():
    _patch_drain()
    nc = bass.Bass()
    dram = {
        "x": nc.dram_tensor("x", [R, D], BF16, kind="ExternalInput"),
        "maskp": nc.dram_tensor("maskp", [128, 16, S], BF16, kind="ExternalInput"),
        "wq": nc.dram_tensor("wq", [128, 8, 4, 2, 128], FP8, kind="ExternalInput"),
        "wk": nc.dram_tensor("wk", [128, 8, 4, 2, 128], FP8, kind="ExternalInput"),
        "wv": nc.dram_tensor("wv", [128, 4, 2, D], FP8, kind="ExternalInput"),
        "bvg": nc.dram_tensor("bvg", [128, 3 * D], BF16, kind="ExternalInput"),
        "cpack": nc.dram_tensor("cpack", [128, 80], F32, kind="ExternalInput"),
        "w1": nc.dram_tensor("w1", [32, 128, 8, 128], BF16, kind="ExternalInput"),
        "w2": nc.dram_tensor("w2", [MLP, D], BF16, kind="ExternalInput"),
        "out": nc.dram_tensor("out", [R, D], F32, kind="ExternalOutput"),
    }
    with tile.TileContext(nc) as tc:
        with tc.tile_pool(name="persist", bufs=1) as pp:
            consts = {}
            consts["x_sb"] = pp.tile([128, 4, D], BF16, tag="x", name="x_sb")
            eps_t = pp.tile([128, 1], F32, tag="eps")
            nc.vector.memset(eps_t, EPS)
            consts["eps"] = eps_t
            consts["cpack"] = pp.tile([128, 80], F32, tag="cpack", name="cpack")
            bvg = pp.tile([128, 3, D], BF16, tag="bvg", name="bvg")
            consts["bvg"] = bvg
            consts["bv_b"] = bvg[:, 0, :]
            consts["g2_b"] = bvg[:, 1, :]
            consts["bcomb_b"] = bvg[:, 2, :]

            with tc.tile_pool(name="poolBC", bufs=1) as pBC:
                xc2 = pBC.tile([128, 4, D], BF16, tag="xc2", name="xc2")
                h2T = pBC.tile([128, 8, R], BF16, tag="h2T", name="h2T")
                with tc.tile_pool(name="poolAB", bufs=1) as pAB:
                    # kblk2: [128, j, s(=e pair idx), tt]; partitions 0:64 hold
                    # even-e K vectors, 64:128 odd-e.
                    consts["kblk2"] = pAB.tile([128, 4, 8, 128], BF16,
                                               tag="kblk2", name="kblk2")
                    # qblk2: [128, j, c, rr], partitions 64:128 duplicate 0:64
                    consts["qblk2"] = pAB.tile([128, 4, 16, 128], BF16,
                                               tag="qblk2", name="qblk2")
                    # vaug: [rr, j, e, d(64) | ones(col 64) | zero pad to 80]
                    vaug = pAB.tile([128, 4, 16, 80], BF16, tag="vaug",
                                    name="vaug")
                    consts["vaug"] = vaug
                    nc.vector.memset(vaug[:, :, :, 64:80], 0.0)
                    nc.vector.memset(vaug[:, :, :, 64:65], 1.0)
                    consts["mask_sb"] = pAB.tile([128, 16, S], BF16, tag="mask",
                                                 name="mask_sb")
                    hattn = pAB.tile([128, 4, D], BF16, tag="hattn",
                                     name="hattn")
                    _phase_a(nc, tc, consts, dram)
                    _phase_b(nc, tc, consts, dram, hattn, xc2, h2T)
                _phase_cd(nc, tc, consts, dram, xc2, h2T)
    n = _split_waits(nc)
    print(f"[kernel] split {n} excess sync-waits onto NoOp carriers")
    return nc


_PROGRAM = None


def _get_program():
    global _PROGRAM
    if _PROGRAM is None:
        _PROGRAM = _build_program()
    return _PROGRAM


def _prep_inputs(x, dis_attn_mask, cls_attn_mask, wq, bq, wk, bk, wv, bv,
                 ln1_g, ln1_b, ln2_g, ln2_b, w1, b1, w2, b2):
    """Host-side prep: per-core shards + weight dtype/layout conversion."""
    x = np.asarray(x, np.float32).astype(BF)

    def wprep_t(w, scale):
        # transposed-proj layout: [p, t8, c, t, m]; din = 256c+128t+p,
        # dout = 128*t8 + m
        arr = (np.asarray(w, np.float32) * scale).astype(F8NP)
        return np.ascontiguousarray(
            arr.reshape(4, 2, 128, 8, 128).transpose(2, 3, 0, 1, 4))

    wq8 = wprep_t(np.asarray(wq, np.float32) / 8.0, WQ_SCALE)
    wk8 = wprep_t(wk, WK_SCALE)
    # natural-proj layout for V: [p, c, t, dout]
    wv8 = np.ascontiguousarray(
        (np.asarray(wv, np.float32) * WV_SCALE).astype(F8NP)
        .reshape(4, 2, 128, D).transpose(2, 0, 1, 3))
    w1_t = np.ascontiguousarray(
        np.asarray(w1, np.float32).astype(BF).reshape(8, 128, 32, 128).transpose(2, 1, 0, 3))
    w2_b = np.asarray(w2, np.float32).astype(BF)
    bcomb = (np.asarray(ln2_b, np.float32) + np.asarray(b2, np.float32))
    bvg = np.ascontiguousarray(np.broadcast_to(
        np.stack([np.asarray(bv, np.float32),
                  np.asarray(ln2_g, np.float32),
                  bcomb])[:, None, :], (3, 128, D)).transpose(1, 0, 2)
        .reshape(128, 3 * D)).astype(BF)

    def pcol(v):
        return np.asarray(v, np.float32).reshape(-1, 128).T

    # b2l folded into the MLP1 bias: gelu input = xc2g@w1 + (b1 + b2l@w1)
    b1m = (np.asarray(b1, np.float32)
           + np.asarray(ln2_b, np.float32) @ np.asarray(w1, np.float32))
    cpack = np.concatenate([
        pcol(np.asarray(bq, np.float32) / 8.0), pcol(bk), pcol(ln1_g),
        pcol(ln1_b), pcol(ln2_g), pcol(ln2_b), pcol(b1m)], axis=1)
    cpack = np.ascontiguousarray(cpack).astype(np.float32)

    masks = {}
    for bb in range(B):
        msum = (np.asarray(dis_attn_mask[bb], np.float32)
                + np.asarray(cls_attn_mask[bb], np.float32))
        maskE = np.exp(
            msum.reshape(128, 16, 128, 16).transpose(3, 2, 1, 0).reshape(S, S))
        # flat per-partition layout [p, eabs, s]: maskE[128*eabs + p, s]
        masks[bb] = np.ascontiguousarray(
            maskE.reshape(16, 128, S).transpose(1, 0, 2)).astype(BF)
    shared = {
        "wq": wq8, "wk": wk8, "wv": wv8, "bvg": bvg, "cpack": cpack,
        "w1": w1_t, "w2": w2_b,
    }
    in_maps = []
    for core in range(NCORES):
        bb = core // 4
        g = core % 4
        m = dict(shared)
        m["x"] = np.ascontiguousarray(x[bb, 512 * g:512 * g + 512])
        m["maskp"] = masks[bb]
        in_maps.append(m)
    return in_maps


def kernel(**inputs):
    nc = _get_program()
    in_maps = _prep_inputs(**inputs)
    res = run_bass_kernel_spmd(nc, in_maps, core_ids=list(range(NCORES)))
    out = np.zeros((B, S, D), np.float32)
    for core in range(NCORES):
        bb = core // 4
        g = core % 4
        out[bb, 512 * g:512 * g + 512] = res.results[core]["out"]
    return out


if __name__ == "__main__":
    sys.path.insert(0, "/root/problem")
    import reference
    inputs = {k: np.asarray(v) for k, v in reference.setup_inputs().items()}
    expected = np.asarray(reference.reference(**inputs))
    actual = kernel(**inputs)
    err = np.abs(actual - expected)
    scale = np.abs(expected).max()
    print("max abs err:", err.max(), "scale:", scale, "rel:", err.max() / scale)
